# revision 3
# baseline (speedup 1.0000x reference)
"""Trainium2 Bass kernel for top-1 MoE (7 routed experts + 1 shared expert).

Contract: kernel(**inputs) takes FULL unsharded inputs (as produced by
setup_inputs) and returns the FULL output, matching reference():
(y [4,2048,1024] f32, aux_loss f32 scalar).

Strategy (8 NeuronCores, SPMD single program):
  - Host computes gate logits/softmax/top-1 (this is the token "dispatch" of
    expert-parallel MoE, done as part of input sharding) and gathers tokens
    by expert.
  - Every core runs the identical program: FFN over R token slots with
    weight set A, then S token slots with weight set B.
      cores 0..6: A = routed expert e (its tokens, gate-scaled), B = shared
      core 7:     A = shared, B = shared
    Shared-expert work is data-parallel across all 8 cores. Pad slots have
    gate 0. The per-token gate is applied on device.
  - All matmul inputs are fp32r (fp32 rounded to s1e8m11, full PE rate,
    ~1.5e-4 relative error). Activations flow transposed ([feature, token])
    so no on-device transposes are needed.
  - Host combines: routed outputs scatter by token index, shared outputs add
    over contiguous ranges.
"""

import math
import os

import numpy as np

import concourse.bass as bass
import concourse.mybir as mybir
import concourse.tile as tile
from concourse import bacc
from concourse.bass_utils import run_bass_kernel_spmd

N_EMBD = 1024
MOE_HIDDEN = 2048
N_ROUTED = 7
N_CORES = 8
GRP = 256  # token-group size (PSUM free dim per matmul)
CT = N_EMBD // 128  # 8 c partition-tiles
HT_HALF = MOE_HIDDEN // 2 // 128  # 8 h partition-tiles per H-half

F32 = mybir.dt.float32
F32R = mybir.dt.float32r

LAST_RESULTS = None  # BassKernelResults of the most recent run (for test.py)
_PLAN_CACHE = {}


def _round_fp32r(x: np.ndarray) -> np.ndarray:
    """Round fp32 to s1e8m11 (fp32r) with round-to-nearest-even."""
    u = np.ascontiguousarray(x, dtype=np.float32).view(np.uint32)
    lsb = (u >> np.uint32(12)) & np.uint32(1)
    r = (u + np.uint32(0x7FF) + lsb) & np.uint32(0xFFFFF000)
    return r.view(np.float32)


def _build_program(R: int, S: int):
    """Build the SPMD Bass program for segment sizes R (weights A) and
    S (weights B); both multiples of GRP. Returns compiled nc."""
    T = R + S
    nc = bacc.Bacc("TRN2", target_bir_lowering=False, debug=False,
                   num_devices=N_CORES)

    xT = nc.dram_tensor("xT", [N_EMBD, T], F32R, kind="ExternalInput").ap()
    gate = nc.dram_tensor("gate", [128, T], F32, kind="ExternalInput").ap()
    wg_a = nc.dram_tensor("wg_a", [N_EMBD, MOE_HIDDEN], F32R, kind="ExternalInput").ap()
    wu_a = nc.dram_tensor("wu_a", [N_EMBD, MOE_HIDDEN], F32R, kind="ExternalInput").ap()
    wd_a = nc.dram_tensor("wd_a", [MOE_HIDDEN, N_EMBD], F32R, kind="ExternalInput").ap()
    wg_b = nc.dram_tensor("wg_b", [N_EMBD, MOE_HIDDEN], F32R, kind="ExternalInput").ap()
    wu_b = nc.dram_tensor("wu_b", [N_EMBD, MOE_HIDDEN], F32R, kind="ExternalInput").ap()
    wd_b = nc.dram_tensor("wd_b", [MOE_HIDDEN, N_EMBD], F32R, kind="ExternalInput").ap()
    yT = nc.dram_tensor("yT", [N_EMBD, T], F32, kind="ExternalOutput").ap()

    segs = []
    if R:
        segs.append((0, R, wg_a, wu_a, wd_a))
    if S:
        segs.append((R, S, wg_b, wu_b, wd_b))

    with tile.TileContext(nc) as tc:
        with (
            tc.tile_pool(name="wpool", bufs=1) as wpool,
            tc.tile_pool(name="xpool", bufs=2) as xpool,
            tc.tile_pool(name="ypool", bufs=1) as ypool,
            tc.tile_pool(name="mpool", bufs=2) as mpool,
            tc.tile_pool(name="spool", bufs=3) as spool,
            tc.tile_pool(name="opool", bufs=4) as opool,
            tc.tile_pool(name="cpool", bufs=1) as cpool,
            tc.tile_pool(name="pg", bufs=2, space="PSUM") as pg,
            tc.tile_pool(name="pu", bufs=2, space="PSUM") as pu,
            tc.tile_pool(name="py", bufs=2, space="PSUM") as py,
        ):
            # --- PE warm-up + ACT table prewarm during initial DMA ---
            warm = cpool.tile([128, 128], F32, tag="warm")
            nc.gpsimd.memset(warm[:], 0.0)
            warm_ps = pg.tile([128, 128], F32, tag="g")
            for i in range(64):
                nc.tensor.matmul(warm_ps[:], warm[:], warm[:],
                                 start=(i == 0), stop=(i == 63))
            warm_silu = cpool.tile([128, 128], F32, tag="warmsilu")
            nc.scalar.activation(warm_silu[:], warm[:],
                                 mybir.ActivationFunctionType.Silu)

            # gate rows resident for the whole kernel
            gate_sb = cpool.tile([128, T], F32, tag="gate")
            nc.sync.dma_start(out=gate_sb[:], in_=gate)

            for col0, Tseg, wg_d, wu_d, wd_d in segs:
                ngrp = Tseg // GRP
                # y accumulator for this segment (per c-tile)
                y_acc = [ypool.tile([128, Tseg], F32, tag=f"y{c}", name=f"yacc{c}")
                         for c in range(CT)]
                for hh in range(2):
                    h0 = hh * (MOE_HIDDEN // 2)
                    # H-half weights
                    wg_sb = [wpool.tile([128, 1024], F32R, tag=f"wg{c}", name=f"wg{c}")
                             for c in range(CT)]
                    wu_sb = [wpool.tile([128, 1024], F32R, tag=f"wu{c}", name=f"wu{c}")
                             for c in range(CT)]
                    wd_sb = [wpool.tile([128, 1024], F32R, tag=f"wd{h}", name=f"wd{h}")
                             for h in range(HT_HALF)]
                    for c in range(CT):
                        nc.sync.dma_start(
                            out=wg_sb[c][:],
                            in_=wg_d[c * 128:(c + 1) * 128, h0:h0 + 1024])
                        nc.sync.dma_start(
                            out=wu_sb[c][:],
                            in_=wu_d[c * 128:(c + 1) * 128, h0:h0 + 1024])
                    for h in range(HT_HALF):
                        nc.sync.dma_start(
                            out=wd_sb[h][:],
                            in_=wd_d[h0 + h * 128:h0 + (h + 1) * 128, :])

                    for g in range(ngrp):
                        cols = bass.ds(col0 + g * GRP, GRP)
                        gcols = bass.ds(g * GRP, GRP)
                        # token group (re-read per hh; keeps SBUF small)
                        xg = [xpool.tile([128, GRP], F32R, tag=f"x{c}", name=f"xg{c}")
                              for c in range(CT)]
                        for c in range(CT):
                            nc.sync.dma_start(
                                out=xg[c][:],
                                in_=xT[c * 128:(c + 1) * 128, cols])

                        multT = []
                        for h in range(HT_HALF):
                            g_ps = pg.tile([128, GRP], F32, tag="g")
                            u_ps = pu.tile([128, GRP], F32, tag="u")
                            for c in range(CT):
                                nc.tensor.matmul(
                                    g_ps[:], wg_sb[c][:, bass.ts(h, 128)],
                                    xg[c][:], start=(c == 0), stop=(c == CT - 1))
                            for c in range(CT):
                                nc.tensor.matmul(
                                    u_ps[:], wu_sb[c][:, bass.ts(h, 128)],
                                    xg[c][:], start=(c == 0), stop=(c == CT - 1))
                            silu_sb = spool.tile([128, GRP], F32, tag="silu")
                            nc.scalar.activation(
                                silu_sb[:], g_ps[:],
                                mybir.ActivationFunctionType.Silu)
                            m = mpool.tile([128, GRP], F32R, tag=f"m{h}")
                            nc.vector.tensor_mul(m[:], silu_sb[:], u_ps[:])
                            multT.append(m)

                        for c in range(CT):
                            y_ps = py.tile([128, GRP], F32, tag="y")
                            for h in range(HT_HALF):
                                nc.tensor.matmul(
                                    y_ps[:], wd_sb[h][:, bass.ts(c, 128)],
                                    multT[h][:], start=(h == 0),
                                    stop=(h == HT_HALF - 1))
                            if hh == 0:
                                # gate applied here; second half adds on top
                                nc.vector.tensor_mul(
                                    y_acc[c][:, gcols], y_ps[:],
                                    gate_sb[:, cols])
                            else:
                                tmp = spool.tile([128, GRP], F32, tag="ytmp")
                                nc.vector.tensor_mul(
                                    tmp[:], y_ps[:], gate_sb[:, cols])
                                o = opool.tile([128, GRP], F32, tag="o")
                                nc.vector.tensor_add(
                                    o[:], y_acc[c][:, gcols], tmp[:])
                                nc.sync.dma_start(
                                    out=yT[c * 128:(c + 1) * 128, cols],
                                    in_=o[:])

    nc.compile()
    return nc


def _get_program(R: int, S: int):
    key = (R, S)
    if key not in _PLAN_CACHE:
        _PLAN_CACHE[key] = _build_program(R, S)
    return _PLAN_CACHE[key]


def kernel(x, gate_w, expert_bias, shared_wg, shared_wu, shared_wd,
           routed_wg, routed_wu, routed_wd):
    x = np.asarray(x, dtype=np.float32)
    gate_w = np.asarray(gate_w, dtype=np.float32)
    expert_bias = np.asarray(expert_bias, dtype=np.float32)
    Bx, Tx, C = x.shape
    N = Bx * Tx
    assert C == N_EMBD
    xf = x.reshape(N, C)

    # ---- routing (host side of the expert-parallel dispatch) ----
    logits = xf.astype(np.float64) @ np.asarray(gate_w, np.float64).T
    m = logits.max(axis=1, keepdims=True)
    e = np.exp(logits - m)
    probs = e / e.sum(axis=1, keepdims=True)  # [N, 7] f64
    top1 = np.argmax(probs + expert_bias[None, :].astype(np.float64), axis=1)
    gate_val = probs[np.arange(N), top1].astype(np.float32)

    counts = np.bincount(top1, minlength=N_ROUTED)
    order = np.argsort(top1, kind="stable")  # tokens grouped by expert
    starts = np.zeros(N_ROUTED + 1, np.int64)
    np.cumsum(counts, out=starts[1:])

    R = max(GRP, int(math.ceil(counts.max() / GRP)) * GRP)
    S = int(math.ceil((N - R) / (N_CORES * GRP))) * GRP if N > R else 0
    T = R + S

    # ---- per-core slot assignment ----
    # idx[core, slot] -> source token; gate per slot; pad slots: idx 0, gate 0
    idx = np.zeros((N_CORES, T), np.int64)
    gates = np.zeros((N_CORES, T), np.float32)
    seg_meta = []  # (core, dst_slice_in_slots, kind, info)
    for e_i in range(N_ROUTED):
        toks = order[starts[e_i]:starts[e_i + 1]]
        idx[e_i, :counts[e_i]] = toks
        gates[e_i, :counts[e_i]] = gate_val[toks]
    # shared: core 7 segment A takes tokens [0:R), segment B on all cores
    sh_a = min(R, N)
    idx[7, :sh_a] = np.arange(sh_a)
    gates[7, :sh_a] = 1.0
    sh_b_counts = []
    pos = sh_a
    for c in range(N_CORES):
        cnt = min(S, N - pos) if pos < N else 0
        if cnt > 0:
            idx[c, R:R + cnt] = np.arange(pos, pos + cnt)
            gates[c, R:R + cnt] = 1.0
        sh_b_counts.append((pos, cnt))
        pos += cnt
    assert pos >= N, f"shared capacity too small: {pos} < {N}"

    # ---- build device inputs ----
    xfT_r = _round_fp32r(np.ascontiguousarray(xf.T))  # [C, N]
    shared_wg_r = _round_fp32r(shared_wg[0])
    shared_wu_r = _round_fp32r(shared_wu[0])
    shared_wd_r = _round_fp32r(shared_wd[0])
    routed_wg_r = _round_fp32r(routed_wg)
    routed_wu_r = _round_fp32r(routed_wu)
    routed_wd_r = _round_fp32r(routed_wd)

    in_maps = []
    for c in range(N_CORES):
        xT_c = np.ascontiguousarray(np.take(xfT_r, idx[c], axis=1))
        gate_c = np.ascontiguousarray(
            np.broadcast_to(gates[c][None, :], (128, T)))
        if c < N_ROUTED:
            wg_a, wu_a, wd_a = routed_wg_r[c], routed_wu_r[c], routed_wd_r[c]
        else:
            wg_a, wu_a, wd_a = shared_wg_r, shared_wu_r, shared_wd_r
        in_maps.append({
            "xT": xT_c, "gate": gate_c,
            "wg_a": wg_a, "wu_a": wu_a, "wd_a": wd_a,
            "wg_b": shared_wg_r, "wu_b": shared_wu_r, "wd_b": shared_wd_r,
        })

    nc = _get_program(R, S)
    res = run_bass_kernel_spmd(nc, in_maps, list(range(N_CORES)))
    global LAST_RESULTS
    LAST_RESULTS = res

    # ---- combine ----
    y = np.zeros((N, C), dtype=np.float32)
    for e_i in range(N_ROUTED):
        cnt = counts[e_i]
        if cnt:
            out_c = res.results[e_i]["yT"]  # [C, T]
            y[idx[e_i, :cnt]] = out_c[:, :cnt].T
    # shared segment A (core 7)
    out7 = res.results[7]["yT"]
    y[:sh_a] += out7[:, :sh_a].T
    # shared segment B
    for c in range(N_CORES):
        pos_c, cnt = sh_b_counts[c]
        if cnt:
            out_c = res.results[c]["yT"]
            y[pos_c:pos_c + cnt] += out_c[:, R:R + cnt].T

    return y.reshape(Bx, Tx, C), np.float32(0.0)


# revision 15
# speedup vs baseline: 1.2583x; 1.2583x over previous
"""Trainium2 Bass kernel for top-1 MoE (7 routed experts + 1 shared expert).

Contract: kernel(**inputs) takes FULL unsharded inputs (as produced by
setup_inputs) and returns the FULL output, matching reference():
(y [4,2048,1024] f32, aux_loss f32 scalar).

Strategy (8 NeuronCores, SPMD single program):
  - Host computes gate logits/softmax/top-1 (the token "dispatch" of
    expert-parallel MoE, done as part of input sharding) and gathers tokens
    by expert.
  - Every core runs the identical program: FFN over R token slots with
    weight set A, then S token slots with weight set B.
      cores 0..6: A = routed expert e (its tokens, gate-scaled), B = shared
      core 7:     A = shared, B = shared
    Shared-expert work is data-parallel across all 8 cores. Pad slots have
    gate 0. The per-token gate is applied on device.
  - All matmul inputs are fp32r (fp32 rounded to s1e8m11, full PE rate,
    ~1.5e-4 relative error). Activations flow transposed ([feature, token])
    through the up-projections; the down-projection is token-stationary so
    the output lands token-major and the gate is a per-partition scalar.
  - Weights stream through SBUF exactly once (H split in halves); y
    accumulates across the two halves in SBUF.
"""

import math
import os

import numpy as np

import concourse.bass as bass
import concourse.mybir as mybir
import concourse.tile as tile
from concourse import bacc
from concourse.bass_utils import run_bass_kernel_spmd

N_EMBD = 1024
MOE_HIDDEN = 2048
N_ROUTED = 7
N_CORES = 8
GRP = int(os.environ.get("MOE_GRP", "256"))  # token-group size
CT = N_EMBD // 128  # 8 c partition-tiles
HT_HALF = MOE_HIDDEN // 2 // 128  # 8 h partition-tiles per H-half

F32 = mybir.dt.float32
F32R = mybir.dt.float32r

LAST_RESULTS = None  # BassKernelResults of the most recent run (for test.py)
_PLAN_CACHE = {}


def _round_fp32r(x: np.ndarray) -> np.ndarray:
    """Round fp32 to s1e8m11 (fp32r) with round-to-nearest-even."""
    u = np.ascontiguousarray(x, dtype=np.float32).view(np.uint32)
    lsb = (u >> np.uint32(12)) & np.uint32(1)
    r = (u + np.uint32(0x7FF) + lsb) & np.uint32(0xFFFFF000)
    return r.view(np.float32)




def _split_blocks(Tseg):
    """Split a segment (multiple of 128) into token blocks of 128-multiples,
    each >= 256 (except a lone 128-segment), mostly 512s."""
    if Tseg <= 512:
        return [Tseg]
    sizes = []
    rem = Tseg
    while rem > 768:
        sizes.append(512)
        rem -= 512
    # rem in (256, 768]
    if rem <= 512:
        sizes.append(rem)
    else:
        sizes.append(rem - 256)
        sizes.append(256)
    return sizes




def _build_program(R: int, S: int):
    """Build the SPMD Bass program for segment sizes R (weights A) and
    S (weights B); both multiples of 256. H is processed in quarters (512)
    so only a quarter of each weight set is SBUF-resident; token blocks of
    512 (plus a 256 remainder) are paired inside each matmul chain so
    consecutive matmuls alternate PSUM banks at N=512."""
    T = R + S
    NS = T // 128  # token subtiles
    HQ = MOE_HIDDEN // 4  # 512 per quarter
    HTQ = HQ // 128  # 4 h-tiles per quarter
    nc = bacc.Bacc("TRN2", target_bir_lowering=False, debug=False,
                   num_devices=N_CORES)

    xT = nc.dram_tensor("xT", [N_EMBD, T], F32R, kind="ExternalInput").ap()
    # gate_cols[p, j] = gate of token j*128+p
    gate = nc.dram_tensor("gate", [128, NS], F32, kind="ExternalInput").ap()
    wg_a = nc.dram_tensor("wg_a", [N_EMBD, MOE_HIDDEN], F32R, kind="ExternalInput").ap()
    wu_a = nc.dram_tensor("wu_a", [N_EMBD, MOE_HIDDEN], F32R, kind="ExternalInput").ap()
    wd_a = nc.dram_tensor("wd_a", [MOE_HIDDEN, N_EMBD], F32R, kind="ExternalInput").ap()
    wg_b = nc.dram_tensor("wg_b", [N_EMBD, MOE_HIDDEN], F32R, kind="ExternalInput").ap()
    wu_b = nc.dram_tensor("wu_b", [N_EMBD, MOE_HIDDEN], F32R, kind="ExternalInput").ap()
    wd_b = nc.dram_tensor("wd_b", [MOE_HIDDEN, N_EMBD], F32R, kind="ExternalInput").ap()
    yO = nc.dram_tensor("yO", [T, N_EMBD], F32, kind="ExternalOutput").ap()

    segs = []
    if R:
        segs.append((0, R, wg_a, wu_a, wd_a))
    if S:
        segs.append((R, S, wg_b, wu_b, wd_b))

    mult_op = mybir.AluOpType.mult
    add_op = mybir.AluOpType.add

    with tile.TileContext(nc) as tc:
        with (
            tc.tile_pool(name="wpool", bufs=1) as wpool,
            tc.tile_pool(name="xpool", bufs=1) as xpool,
            tc.tile_pool(name="ypool", bufs=1) as ypool,
            tc.tile_pool(name="mpool", bufs=14) as mpool,
            tc.tile_pool(name="spool", bufs=3) as spool,
            tc.tile_pool(name="opool", bufs=3) as opool,
            tc.tile_pool(name="cpool", bufs=1) as cpool,
            tc.tile_pool(name="pg", bufs=1, space="PSUM") as pg,
            tc.tile_pool(name="pu", bufs=1, space="PSUM") as pu,
            tc.tile_pool(name="py", bufs=1, space="PSUM") as py,
        ):
            # --- PE warm-up + ACT table prewarm during initial DMA ---
            warm = cpool.tile([128, 128], F32, tag="warm")
            nc.gpsimd.memset(warm[:], 0.0)
            warm_ps = pg.tile([128, 128], F32, tag="g0", name="warm_ps")
            for i in range(56):
                nc.tensor.matmul(warm_ps[:], warm[:], warm[:],
                                 start=(i == 0), stop=(i == 55))
            warm_silu = cpool.tile([128, 128], F32, tag="warmsilu")
            nc.scalar.activation(warm_silu[:], warm[:],
                                 mybir.ActivationFunctionType.Silu)

            gate_sb = cpool.tile([128, NS], F32, tag="gate")
            first_dma = [True]

            for col0, Tseg, wg_d, wu_d, wd_d in segs:
                bsizes = _split_blocks(Tseg)
                boffs = [sum(bsizes[:i]) for i in range(len(bsizes))]
                nb = len(bsizes)
                # pair consecutive blocks (shared stationary, alternating
                # PSUM banks); a leftover block runs solo
                pairs = [(i, i + 1) for i in range(0, nb - 1, 2)]
                solos = [nb - 1] if nb % 2 else []

                # resident token blocks for this segment
                x_sb = [[xpool.tile([128, bsizes[j]], F32R, tag=f"x{c}b{j}",
                                    name=f"x{c}b{j}", padded_shape=[128, 512])
                         for j in range(nb)] for c in range(CT)]
                # y accumulator (per token subtile)
                y_acc = [ypool.tile([128, N_EMBD], F32, tag=f"ya{j}",
                                    name=f"yacc{j}")
                         for j in range(Tseg // 128)]

                def load_x(js):
                    for c in range(CT):
                        for j in js:
                            nc.sync.dma_start(
                                out=x_sb[c][j][:],
                                in_=xT[c * 128:(c + 1) * 128,
                                       bass.ds(col0 + boffs[j], bsizes[j])])

                for q in range(4):
                    h0 = q * HQ
                    with nc.named_scope(f"s{col0 > 0:d}q{q}"):
                        wg_sb = [wpool.tile([128, HQ], F32R, tag=f"wg{c}",
                                            name=f"wg{c}") for c in range(CT)]
                        wu_sb = [wpool.tile([128, HQ], F32R, tag=f"wu{c}",
                                            name=f"wu{c}") for c in range(CT)]
                        wd_sb = [wpool.tile([128, N_EMBD], F32R, tag=f"wd{h}",
                                            name=f"wd{h}") for h in range(HTQ)]
                        for c in range(CT):
                            nc.sync.dma_start(
                                out=wg_sb[c][:],
                                in_=wg_d[c * 128:(c + 1) * 128, h0:h0 + HQ])
                        if q == 0:
                            load_x(range(min(nb, 2)))
                        for c in range(CT):
                            nc.sync.dma_start(
                                out=wu_sb[c][:],
                                in_=wu_d[c * 128:(c + 1) * 128, h0:h0 + HQ])
                        for h in range(HTQ):
                            nc.sync.dma_start(
                                out=wd_sb[h][:],
                                in_=wd_d[h0 + h * 128:h0 + (h + 1) * 128, :])
                        if q == 0:
                            load_x(range(2, nb))
                        if first_dma[0]:
                            nc.sync.dma_start(out=gate_sb[:], in_=gate)
                            first_dma[0] = False

                        def yphase(j, mult):
                            # token-stationary down-projection for block j
                            for t in range(bsizes[j] // 128):
                                st = (col0 + boffs[j]) // 128 + t
                                sl = st - col0 // 128
                                y_ps = py.tile([128, N_EMBD], F32,
                                               tag=f"yt{t % 2}",
                                               name=f"yps{t % 2}")
                                for h in range(HTQ):
                                    lhsT = mult[h][:, bass.ts(t, 128)]
                                    for half in range(2):
                                        nc.tensor.matmul(
                                            y_ps[:, bass.ts(half, 512)],
                                            lhsT,
                                            wd_sb[h][:, bass.ts(half, 512)],
                                            start=(h == 0),
                                            stop=(h == HTQ - 1))
                                gate_ap = gate_sb[:, st:st + 1]
                                if q == 0:
                                    nc.vector.tensor_scalar_mul(
                                        y_acc[sl][:], y_ps[:], gate_ap)
                                elif q == 3:
                                    o = opool.tile([128, N_EMBD], F32, tag="o")
                                    nc.vector.scalar_tensor_tensor(
                                        o[:], y_ps[:], gate_ap, y_acc[sl][:],
                                        mult_op, add_op)
                                    nc.sync.dma_start(
                                        out=yO[st * 128:(st + 1) * 128, :],
                                        in_=o[:])
                                else:
                                    nc.vector.scalar_tensor_tensor(
                                        y_acc[sl][:], y_ps[:], gate_ap,
                                        y_acc[sl][:], mult_op, add_op)

                        def mults_for(blocks, h):
                            # g/u chains for 1 or 2 blocks; stationary shared
                            # across the pair so N=512 matmuls alternate banks
                            gps = [pg.tile([128, bsizes[j]], F32,
                                           tag=f"g{k}", name=f"g{k}",
                                           padded_shape=[128, 512])
                                   for k, j in enumerate(blocks)]
                            ups = [pu.tile([128, bsizes[j]], F32,
                                           tag=f"u{k}", name=f"u{k}",
                                           padded_shape=[128, 512])
                                   for k, j in enumerate(blocks)]
                            for c in range(CT):
                                for k, j in enumerate(blocks):
                                    nc.tensor.matmul(
                                        gps[k][:], wg_sb[c][:, bass.ts(h, 128)],
                                        x_sb[c][j][:], start=(c == 0),
                                        stop=(c == CT - 1))
                            for c in range(CT):
                                for k, j in enumerate(blocks):
                                    nc.tensor.matmul(
                                        ups[k][:], wu_sb[c][:, bass.ts(h, 128)],
                                        x_sb[c][j][:], start=(c == 0),
                                        stop=(c == CT - 1))
                            out = []
                            for k, j in enumerate(blocks):
                                silu_sb = spool.tile([128, bsizes[j]], F32,
                                                     tag="silu", name="silu_sb",
                                                     padded_shape=[128, 512])
                                nc.scalar.activation(
                                    silu_sb[:], gps[k][:],
                                    mybir.ActivationFunctionType.Silu)
                                m = mpool.tile([128, bsizes[j]], F32R, tag="m",
                                               name=f"m{h}",
                                               padded_shape=[128, 512])
                                nc.vector.tensor_mul(m[:], silu_sb[:], ups[k][:])
                                out.append(m)
                            return out

                        for jA, jB in pairs:
                            multA, multB = [], []
                            for h in range(HTQ):
                                ms = mults_for((jA, jB), h)
                                multA.append(ms[0])
                                multB.append(ms[1])
                            yphase(jA, multA)
                            yphase(jB, multB)
                        for j in solos:
                            multS = []
                            for h in range(HTQ):
                                multS.append(mults_for((j,), h)[0])
                            yphase(j, multS)

    nc.compile()
    return nc


def _get_program(R: int, S: int):
    key = (R, S)
    if key not in _PLAN_CACHE:
        _PLAN_CACHE[key] = _build_program(R, S)
    return _PLAN_CACHE[key]


def kernel(x, gate_w, expert_bias, shared_wg, shared_wu, shared_wd,
           routed_wg, routed_wu, routed_wd):
    x = np.asarray(x, dtype=np.float32)
    gate_w = np.asarray(gate_w, dtype=np.float32)
    expert_bias = np.asarray(expert_bias, dtype=np.float32)
    Bx, Tx, C = x.shape
    N = Bx * Tx
    assert C == N_EMBD
    xf = x.reshape(N, C)

    # ---- routing (host side of the expert-parallel dispatch) ----
    logits = xf.astype(np.float64) @ np.asarray(gate_w, np.float64).T
    m = logits.max(axis=1, keepdims=True)
    e = np.exp(logits - m)
    probs = e / e.sum(axis=1, keepdims=True)  # [N, 7] f64
    top1 = np.argmax(probs + expert_bias[None, :].astype(np.float64), axis=1)
    gate_val = probs[np.arange(N), top1].astype(np.float32)

    counts = np.bincount(top1, minlength=N_ROUTED)
    order = np.argsort(top1, kind="stable")  # tokens grouped by expert
    starts = np.zeros(N_ROUTED + 1, np.int64)
    np.cumsum(counts, out=starts[1:])

    R = max(256, int(math.ceil(counts.max() / 128)) * 128)
    S = int(math.ceil((N - R) / (N_CORES * 128))) * 128 if N > R else 0
    T = R + S

    # ---- per-core slot assignment ----
    # idx[core, slot] -> source token; gate per slot; pad slots: idx 0, gate 0
    idx = np.zeros((N_CORES, T), np.int64)
    gates = np.zeros((N_CORES, T), np.float32)
    for e_i in range(N_ROUTED):
        toks = order[starts[e_i]:starts[e_i + 1]]
        idx[e_i, :counts[e_i]] = toks
        gates[e_i, :counts[e_i]] = gate_val[toks]
    # shared: core 7 segment A takes tokens [0:R), segment B on all cores
    sh_a = min(R, N)
    idx[7, :sh_a] = np.arange(sh_a)
    gates[7, :sh_a] = 1.0
    sh_b_counts = []
    pos = sh_a
    for c in range(N_CORES):
        cnt = min(S, N - pos) if pos < N else 0
        if cnt > 0:
            idx[c, R:R + cnt] = np.arange(pos, pos + cnt)
            gates[c, R:R + cnt] = 1.0
        sh_b_counts.append((pos, cnt))
        pos += cnt
    assert pos >= N, f"shared capacity too small: {pos} < {N}"

    # ---- build device inputs ----
    xfT_r = _round_fp32r(np.ascontiguousarray(xf.T))  # [C, N]
    shared_wg_r = _round_fp32r(shared_wg[0])
    shared_wu_r = _round_fp32r(shared_wu[0])
    shared_wd_r = _round_fp32r(shared_wd[0])
    routed_wg_r = _round_fp32r(routed_wg)
    routed_wu_r = _round_fp32r(routed_wu)
    routed_wd_r = _round_fp32r(routed_wd)

    in_maps = []
    for c in range(N_CORES):
        xT_c = np.ascontiguousarray(np.take(xfT_r, idx[c], axis=1))
        gate_c = np.ascontiguousarray(gates[c].reshape(T // 128, 128).T)
        if c < N_ROUTED:
            wg_a, wu_a, wd_a = routed_wg_r[c], routed_wu_r[c], routed_wd_r[c]
        else:
            wg_a, wu_a, wd_a = shared_wg_r, shared_wu_r, shared_wd_r
        in_maps.append({
            "xT": xT_c, "gate": gate_c,
            "wg_a": wg_a, "wu_a": wu_a, "wd_a": wd_a,
            "wg_b": shared_wg_r, "wu_b": shared_wu_r, "wd_b": shared_wd_r,
        })

    nc = _get_program(R, S)
    res = run_bass_kernel_spmd(nc, in_maps, list(range(N_CORES)))
    global LAST_RESULTS
    LAST_RESULTS = res

    # ---- combine (outputs are token-major [T, C]) ----
    y = np.zeros((N, C), dtype=np.float32)
    for e_i in range(N_ROUTED):
        cnt = counts[e_i]
        if cnt:
            out_c = res.results[e_i]["yO"]  # [T, C]
            y[idx[e_i, :cnt]] = out_c[:cnt]
    # shared segment A (core 7)
    out7 = res.results[7]["yO"]
    y[:sh_a] += out7[:sh_a]
    # shared segment B
    for c in range(N_CORES):
        pos_c, cnt = sh_b_counts[c]
        if cnt:
            out_c = res.results[c]["yO"]
            y[pos_c:pos_c + cnt] += out_c[R:R + cnt]

    return y.reshape(Bx, Tx, C), np.float32(0.0)


# revision 17
# speedup vs baseline: 1.2907x; 1.0258x over previous
"""Trainium2 Bass kernel for top-1 MoE (7 routed experts + 1 shared expert).

Contract: kernel(**inputs) takes FULL unsharded inputs (as produced by
setup_inputs) and returns the FULL output, matching reference():
(y [4,2048,1024] f32, aux_loss f32 scalar).

Strategy (8 NeuronCores, SPMD single program):
  - Host computes gate logits/softmax/top-1 (the token "dispatch" of
    expert-parallel MoE, done as part of input sharding) and gathers tokens
    by expert.
  - Every core runs the identical program: FFN over R token slots with
    weight set A, then S token slots with weight set B.
      cores 0..6: A = routed expert e (its tokens, gate-scaled), B = shared
      core 7:     A = shared, B = shared
    Shared-expert work is data-parallel across all 8 cores. Pad slots have
    gate 0. The per-token gate is applied on device.
  - All matmul inputs are fp32r (fp32 rounded to s1e8m11, full PE rate,
    ~1.5e-4 relative error). Activations flow transposed ([feature, token])
    through the up-projections; the down-projection is token-stationary so
    the output lands token-major and the gate is a per-partition scalar.
  - Weights stream through SBUF exactly once (H split in halves); y
    accumulates across the two halves in SBUF.
"""

import math
import os

import numpy as np

import concourse.bass as bass
import concourse.mybir as mybir
import concourse.tile as tile
from concourse import bacc
from concourse.bass_utils import run_bass_kernel_spmd

N_EMBD = 1024
MOE_HIDDEN = 2048
N_ROUTED = 7
N_CORES = 8
GRP = int(os.environ.get("MOE_GRP", "256"))  # token-group size
CT = N_EMBD // 128  # 8 c partition-tiles
HT_HALF = MOE_HIDDEN // 2 // 128  # 8 h partition-tiles per H-half

F32 = mybir.dt.float32
F32R = mybir.dt.float32r

LAST_RESULTS = None  # BassKernelResults of the most recent run (for test.py)
_PLAN_CACHE = {}


def _round_fp32r(x: np.ndarray) -> np.ndarray:
    """Round fp32 to s1e8m11 (fp32r) with round-to-nearest-even."""
    u = np.ascontiguousarray(x, dtype=np.float32).view(np.uint32)
    lsb = (u >> np.uint32(12)) & np.uint32(1)
    r = (u + np.uint32(0x7FF) + lsb) & np.uint32(0xFFFFF000)
    return r.view(np.float32)




def _split_blocks(Tseg):
    """Split a segment (multiple of 128) into token blocks of 128-multiples,
    each >= 256 (except a lone 128-segment), mostly 512s."""
    if Tseg <= 512:
        return [Tseg]
    sizes = []
    rem = Tseg
    while rem > 768:
        sizes.append(512)
        rem -= 512
    # rem in (256, 768]
    if rem <= 512:
        sizes.append(rem)
    else:
        sizes.append(rem - 256)
        sizes.append(256)
    return sizes




def _build_program(R: int, S: int):
    """Build the SPMD Bass program for segment sizes R (weights A) and
    S (weights B); both multiples of 256. H is processed in quarters (512)
    so only a quarter of each weight set is SBUF-resident; token blocks of
    512 (plus a 256 remainder) are paired inside each matmul chain so
    consecutive matmuls alternate PSUM banks at N=512."""
    T = R + S
    NS = T // 128  # token subtiles
    HQ = MOE_HIDDEN // 4  # 512 per quarter
    HTQ = HQ // 128  # 4 h-tiles per quarter
    nc = bacc.Bacc("TRN2", target_bir_lowering=False, debug=False,
                   num_devices=N_CORES)

    xT = nc.dram_tensor("xT", [N_EMBD, T], F32R, kind="ExternalInput").ap()
    # gate_cols[p, j] = gate of token j*128+p
    gate = nc.dram_tensor("gate", [128, NS], F32, kind="ExternalInput").ap()
    wg_a = nc.dram_tensor("wg_a", [N_EMBD, MOE_HIDDEN], F32R, kind="ExternalInput").ap()
    wu_a = nc.dram_tensor("wu_a", [N_EMBD, MOE_HIDDEN], F32R, kind="ExternalInput").ap()
    wd_a = nc.dram_tensor("wd_a", [MOE_HIDDEN, N_EMBD], F32R, kind="ExternalInput").ap()
    wg_b = nc.dram_tensor("wg_b", [N_EMBD, MOE_HIDDEN], F32R, kind="ExternalInput").ap()
    wu_b = nc.dram_tensor("wu_b", [N_EMBD, MOE_HIDDEN], F32R, kind="ExternalInput").ap()
    wd_b = nc.dram_tensor("wd_b", [MOE_HIDDEN, N_EMBD], F32R, kind="ExternalInput").ap()
    yO = nc.dram_tensor("yO", [T, N_EMBD], F32, kind="ExternalOutput").ap()

    segs = []
    if R:
        segs.append((0, R, wg_a, wu_a, wd_a))
    if S:
        segs.append((R, S, wg_b, wu_b, wd_b))

    mult_op = mybir.AluOpType.mult
    add_op = mybir.AluOpType.add

    with tile.TileContext(nc) as tc:
        with (
            tc.tile_pool(name="wpool", bufs=1) as wpool,
            tc.tile_pool(name="xpool", bufs=1) as xpool,
            tc.tile_pool(name="ypool", bufs=1) as ypool,
            tc.tile_pool(name="mpool", bufs=14) as mpool,
            tc.tile_pool(name="spool", bufs=3) as spool,
            tc.tile_pool(name="opool", bufs=3) as opool,
            tc.tile_pool(name="cpool", bufs=1) as cpool,
            tc.tile_pool(name="pg", bufs=1, space="PSUM") as pg,
            tc.tile_pool(name="pu", bufs=1, space="PSUM") as pu,
            tc.tile_pool(name="py", bufs=1, space="PSUM") as py,
        ):
            # --- PE warm-up + ACT table prewarm during initial DMA ---
            warm = cpool.tile([128, 128], F32, tag="warm")
            nc.gpsimd.memset(warm[:], 0.0)
            warm_ps = pg.tile([128, 128], F32, tag="g0", name="warm_ps")
            for i in range(56):
                nc.tensor.matmul(warm_ps[:], warm[:], warm[:],
                                 start=(i == 0), stop=(i == 55))
            warm_silu = cpool.tile([128, 128], F32, tag="warmsilu")
            nc.scalar.activation(warm_silu[:], warm[:],
                                 mybir.ActivationFunctionType.Silu)

            gate_sb = cpool.tile([128, NS], F32, tag="gate")
            first_dma = [True]

            for col0, Tseg, wg_d, wu_d, wd_d in segs:
                bsizes = _split_blocks(Tseg)
                boffs = [sum(bsizes[:i]) for i in range(len(bsizes))]
                nb = len(bsizes)
                # pair consecutive blocks (shared stationary, alternating
                # PSUM banks); a leftover block runs solo
                pairs = [(i, i + 1) for i in range(0, nb - 1, 2)]
                solos = [nb - 1] if nb % 2 else []

                # resident token blocks for this segment
                x_sb = [[xpool.tile([128, bsizes[j]], F32R, tag=f"x{c}b{j}",
                                    name=f"x{c}b{j}", padded_shape=[128, 512])
                         for j in range(nb)] for c in range(CT)]
                # y accumulator (per token subtile)
                y_acc = [ypool.tile([128, N_EMBD], F32, tag=f"ya{j}",
                                    name=f"yacc{j}")
                         for j in range(Tseg // 128)]

                def load_x(js):
                    for c in range(CT):
                        for j in js:
                            nc.sync.dma_start(
                                out=x_sb[c][j][:],
                                in_=xT[c * 128:(c + 1) * 128,
                                       bass.ds(col0 + boffs[j], bsizes[j])])

                for q in range(4):
                    h0 = q * HQ
                    with nc.named_scope(f"s{col0 > 0:d}q{q}"):
                        wg_sb = [wpool.tile([128, HQ], F32R, tag=f"wg{c}",
                                            name=f"wg{c}", bufs=2)
                                 for c in range(CT)]
                        wu_sb = [wpool.tile([128, HQ], F32R, tag=f"wu{c}",
                                            name=f"wu{c}") for c in range(CT)]
                        wd_sb = [wpool.tile([128, N_EMBD], F32R, tag=f"wd{h}",
                                            name=f"wd{h}") for h in range(HTQ)]
                        for c in range(CT):
                            nc.sync.dma_start(
                                out=wg_sb[c][:],
                                in_=wg_d[c * 128:(c + 1) * 128, h0:h0 + HQ])
                        if q == 0:
                            load_x(range(min(nb, 2)))
                        for c in range(CT):
                            nc.sync.dma_start(
                                out=wu_sb[c][:],
                                in_=wu_d[c * 128:(c + 1) * 128, h0:h0 + HQ])
                        for h in range(HTQ):
                            nc.sync.dma_start(
                                out=wd_sb[h][:],
                                in_=wd_d[h0 + h * 128:h0 + (h + 1) * 128, :])
                        if q == 0:
                            load_x(range(2, nb))
                        if first_dma[0]:
                            nc.sync.dma_start(out=gate_sb[:], in_=gate)
                            first_dma[0] = False

                        def yphase(j, mult):
                            # token-stationary down-projection for block j
                            for t in range(bsizes[j] // 128):
                                st = (col0 + boffs[j]) // 128 + t
                                sl = st - col0 // 128
                                y_ps = py.tile([128, N_EMBD], F32,
                                               tag=f"yt{t % 2}",
                                               name=f"yps{t % 2}")
                                for h in range(HTQ):
                                    lhsT = mult[h][:, bass.ts(t, 128)]
                                    for half in range(2):
                                        nc.tensor.matmul(
                                            y_ps[:, bass.ts(half, 512)],
                                            lhsT,
                                            wd_sb[h][:, bass.ts(half, 512)],
                                            start=(h == 0),
                                            stop=(h == HTQ - 1))
                                gate_ap = gate_sb[:, st:st + 1]
                                if q == 0:
                                    nc.vector.tensor_scalar_mul(
                                        y_acc[sl][:], y_ps[:], gate_ap)
                                elif q == 3:
                                    o = opool.tile([128, N_EMBD], F32, tag="o")
                                    nc.vector.scalar_tensor_tensor(
                                        o[:], y_ps[:], gate_ap, y_acc[sl][:],
                                        mult_op, add_op)
                                    nc.sync.dma_start(
                                        out=yO[st * 128:(st + 1) * 128, :],
                                        in_=o[:])
                                else:
                                    nc.vector.scalar_tensor_tensor(
                                        y_acc[sl][:], y_ps[:], gate_ap,
                                        y_acc[sl][:], mult_op, add_op)

                        def mults_for(blocks, h):
                            # g/u chains for 1 or 2 blocks; stationary shared
                            # across the pair so N=512 matmuls alternate banks
                            gps = [pg.tile([128, bsizes[j]], F32,
                                           tag=f"g{k}", name=f"g{k}",
                                           padded_shape=[128, 512])
                                   for k, j in enumerate(blocks)]
                            ups = [pu.tile([128, bsizes[j]], F32,
                                           tag=f"u{k}", name=f"u{k}",
                                           padded_shape=[128, 512])
                                   for k, j in enumerate(blocks)]
                            for c in range(CT):
                                for k, j in enumerate(blocks):
                                    nc.tensor.matmul(
                                        gps[k][:], wg_sb[c][:, bass.ts(h, 128)],
                                        x_sb[c][j][:], start=(c == 0),
                                        stop=(c == CT - 1))
                            for c in range(CT):
                                for k, j in enumerate(blocks):
                                    nc.tensor.matmul(
                                        ups[k][:], wu_sb[c][:, bass.ts(h, 128)],
                                        x_sb[c][j][:], start=(c == 0),
                                        stop=(c == CT - 1))
                            out = []
                            for k, j in enumerate(blocks):
                                silu_sb = spool.tile([128, bsizes[j]], F32,
                                                     tag="silu", name="silu_sb",
                                                     padded_shape=[128, 512])
                                nc.scalar.activation(
                                    silu_sb[:], gps[k][:],
                                    mybir.ActivationFunctionType.Silu)
                                m = mpool.tile([128, bsizes[j]], F32R, tag="m",
                                               name=f"m{h}",
                                               padded_shape=[128, 512])
                                nc.vector.tensor_mul(m[:], silu_sb[:], ups[k][:])
                                out.append(m)
                            return out

                        for jA, jB in pairs:
                            multA, multB = [], []
                            for h in range(HTQ):
                                ms = mults_for((jA, jB), h)
                                multA.append(ms[0])
                                multB.append(ms[1])
                            yphase(jA, multA)
                            yphase(jB, multB)
                        for j in solos:
                            multS = []
                            for h in range(HTQ):
                                multS.append(mults_for((j,), h)[0])
                            yphase(j, multS)

    nc.compile()
    return nc


def _get_program(R: int, S: int):
    key = (R, S)
    if key not in _PLAN_CACHE:
        _PLAN_CACHE[key] = _build_program(R, S)
    return _PLAN_CACHE[key]


def kernel(x, gate_w, expert_bias, shared_wg, shared_wu, shared_wd,
           routed_wg, routed_wu, routed_wd):
    x = np.asarray(x, dtype=np.float32)
    gate_w = np.asarray(gate_w, dtype=np.float32)
    expert_bias = np.asarray(expert_bias, dtype=np.float32)
    Bx, Tx, C = x.shape
    N = Bx * Tx
    assert C == N_EMBD
    xf = x.reshape(N, C)

    # ---- routing (host side of the expert-parallel dispatch) ----
    logits = xf.astype(np.float64) @ np.asarray(gate_w, np.float64).T
    m = logits.max(axis=1, keepdims=True)
    e = np.exp(logits - m)
    probs = e / e.sum(axis=1, keepdims=True)  # [N, 7] f64
    top1 = np.argmax(probs + expert_bias[None, :].astype(np.float64), axis=1)
    gate_val = probs[np.arange(N), top1].astype(np.float32)

    counts = np.bincount(top1, minlength=N_ROUTED)
    order = np.argsort(top1, kind="stable")  # tokens grouped by expert
    starts = np.zeros(N_ROUTED + 1, np.int64)
    np.cumsum(counts, out=starts[1:])

    R = max(256, int(math.ceil(counts.max() / 128)) * 128)
    S = int(math.ceil((N - R) / (N_CORES * 128))) * 128 if N > R else 0
    T = R + S

    # ---- per-core slot assignment ----
    # idx[core, slot] -> source token; gate per slot; pad slots: idx 0, gate 0
    idx = np.zeros((N_CORES, T), np.int64)
    gates = np.zeros((N_CORES, T), np.float32)
    for e_i in range(N_ROUTED):
        toks = order[starts[e_i]:starts[e_i + 1]]
        idx[e_i, :counts[e_i]] = toks
        gates[e_i, :counts[e_i]] = gate_val[toks]
    # shared: core 7 segment A takes tokens [0:R), segment B on all cores
    sh_a = min(R, N)
    idx[7, :sh_a] = np.arange(sh_a)
    gates[7, :sh_a] = 1.0
    sh_b_counts = []
    pos = sh_a
    for c in range(N_CORES):
        cnt = min(S, N - pos) if pos < N else 0
        if cnt > 0:
            idx[c, R:R + cnt] = np.arange(pos, pos + cnt)
            gates[c, R:R + cnt] = 1.0
        sh_b_counts.append((pos, cnt))
        pos += cnt
    assert pos >= N, f"shared capacity too small: {pos} < {N}"

    # ---- build device inputs ----
    xfT_r = _round_fp32r(np.ascontiguousarray(xf.T))  # [C, N]
    shared_wg_r = _round_fp32r(shared_wg[0])
    shared_wu_r = _round_fp32r(shared_wu[0])
    shared_wd_r = _round_fp32r(shared_wd[0])
    routed_wg_r = _round_fp32r(routed_wg)
    routed_wu_r = _round_fp32r(routed_wu)
    routed_wd_r = _round_fp32r(routed_wd)

    in_maps = []
    for c in range(N_CORES):
        xT_c = np.ascontiguousarray(np.take(xfT_r, idx[c], axis=1))
        gate_c = np.ascontiguousarray(gates[c].reshape(T // 128, 128).T)
        if c < N_ROUTED:
            wg_a, wu_a, wd_a = routed_wg_r[c], routed_wu_r[c], routed_wd_r[c]
        else:
            wg_a, wu_a, wd_a = shared_wg_r, shared_wu_r, shared_wd_r
        in_maps.append({
            "xT": xT_c, "gate": gate_c,
            "wg_a": wg_a, "wu_a": wu_a, "wd_a": wd_a,
            "wg_b": shared_wg_r, "wu_b": shared_wu_r, "wd_b": shared_wd_r,
        })

    nc = _get_program(R, S)
    try:
        res = run_bass_kernel_spmd(nc, in_maps, list(range(N_CORES)))
    except Exception:
        # transient NRT exec faults have been observed; retry once
        import time as _time
        _time.sleep(5.0)
        res = run_bass_kernel_spmd(nc, in_maps, list(range(N_CORES)))
    global LAST_RESULTS
    LAST_RESULTS = res

    # ---- combine (outputs are token-major [T, C]) ----
    y = np.zeros((N, C), dtype=np.float32)
    for e_i in range(N_ROUTED):
        cnt = counts[e_i]
        if cnt:
            out_c = res.results[e_i]["yO"]  # [T, C]
            y[idx[e_i, :cnt]] = out_c[:cnt]
    # shared segment A (core 7)
    out7 = res.results[7]["yO"]
    y[:sh_a] += out7[:sh_a]
    # shared segment B
    for c in range(N_CORES):
        pos_c, cnt = sh_b_counts[c]
        if cnt:
            out_c = res.results[c]["yO"]
            y[pos_c:pos_c + cnt] += out_c[R:R + cnt]

    return y.reshape(Bx, Tx, C), np.float32(0.0)


# revision 19
# speedup vs baseline: 1.3007x; 1.0077x over previous
"""Trainium2 Bass kernel for top-1 MoE (7 routed experts + 1 shared expert).

Contract: kernel(**inputs) takes FULL unsharded inputs (as produced by
setup_inputs) and returns the FULL output, matching reference():
(y [4,2048,1024] f32, aux_loss f32 scalar).

Strategy (8 NeuronCores, SPMD single program):
  - Host computes gate logits/softmax/top-1 (the token "dispatch" of
    expert-parallel MoE, done as part of input sharding) and gathers tokens
    by expert.
  - Every core runs the identical program: FFN over R token slots with
    weight set A, then S token slots with weight set B.
      cores 0..6: A = routed expert e (its tokens, gate-scaled), B = shared
      core 7:     A = shared, B = shared
    Shared-expert work is data-parallel across all 8 cores. Pad slots have
    gate 0. The per-token gate is applied on device.
  - All matmul inputs are fp32r (fp32 rounded to s1e8m11, full PE rate,
    ~1.5e-4 relative error). Activations flow transposed ([feature, token])
    through the up-projections; the down-projection is token-stationary so
    the output lands token-major and the gate is a per-partition scalar.
  - Weights stream through SBUF exactly once (H split in halves); y
    accumulates across the two halves in SBUF.
"""

import math
import os

import numpy as np

import concourse.bass as bass
import concourse.mybir as mybir
import concourse.tile as tile
from concourse import bacc
from concourse.bass_utils import run_bass_kernel_spmd

N_EMBD = 1024
MOE_HIDDEN = 2048
N_ROUTED = 7
N_CORES = 8
GRP = int(os.environ.get("MOE_GRP", "256"))  # token-group size
CT = N_EMBD // 128  # 8 c partition-tiles
HT_HALF = MOE_HIDDEN // 2 // 128  # 8 h partition-tiles per H-half

F32 = mybir.dt.float32
F32R = mybir.dt.float32r

LAST_RESULTS = None  # BassKernelResults of the most recent run (for test.py)
_PLAN_CACHE = {}


def _round_fp32r(x: np.ndarray) -> np.ndarray:
    """Round fp32 to s1e8m11 (fp32r) with round-to-nearest-even."""
    u = np.ascontiguousarray(x, dtype=np.float32).view(np.uint32)
    lsb = (u >> np.uint32(12)) & np.uint32(1)
    r = (u + np.uint32(0x7FF) + lsb) & np.uint32(0xFFFFF000)
    return r.view(np.float32)




def _split_blocks(Tseg):
    """Split a segment (multiple of 128) into token blocks of 128-multiples,
    each >= 256 (except a lone 128-segment), mostly 512s."""
    if Tseg <= 512:
        return [Tseg]
    sizes = []
    rem = Tseg
    while rem > 768:
        sizes.append(512)
        rem -= 512
    # rem in (256, 768]
    if rem <= 512:
        sizes.append(rem)
    else:
        sizes.append(rem - 256)
        sizes.append(256)
    return sizes




def _build_program(R: int, S: int):
    """Build the SPMD Bass program for segment sizes R (weights A) and
    S (weights B); both multiples of 256. H is processed in quarters (512)
    so only a quarter of each weight set is SBUF-resident; token blocks of
    512 (plus a 256 remainder) are paired inside each matmul chain so
    consecutive matmuls alternate PSUM banks at N=512."""
    T = R + S
    NS = T // 128  # token subtiles
    HQ = MOE_HIDDEN // 4  # 512 per quarter
    HTQ = HQ // 128  # 4 h-tiles per quarter
    nc = bacc.Bacc("TRN2", target_bir_lowering=False, debug=False,
                   num_devices=N_CORES)

    xT = nc.dram_tensor("xT", [N_EMBD, T], F32R, kind="ExternalInput").ap()
    # gate_cols[p, j] = gate of token j*128+p
    gate = nc.dram_tensor("gate", [128, NS], F32, kind="ExternalInput").ap()
    wg_a = nc.dram_tensor("wg_a", [N_EMBD, MOE_HIDDEN], F32R, kind="ExternalInput").ap()
    wu_a = nc.dram_tensor("wu_a", [N_EMBD, MOE_HIDDEN], F32R, kind="ExternalInput").ap()
    wd_a = nc.dram_tensor("wd_a", [MOE_HIDDEN, N_EMBD], F32R, kind="ExternalInput").ap()
    wg_b = nc.dram_tensor("wg_b", [N_EMBD, MOE_HIDDEN], F32R, kind="ExternalInput").ap()
    wu_b = nc.dram_tensor("wu_b", [N_EMBD, MOE_HIDDEN], F32R, kind="ExternalInput").ap()
    wd_b = nc.dram_tensor("wd_b", [MOE_HIDDEN, N_EMBD], F32R, kind="ExternalInput").ap()
    yO = nc.dram_tensor("yO", [T, N_EMBD], F32, kind="ExternalOutput").ap()

    segs = []
    if R:
        segs.append((0, R, wg_a, wu_a, wd_a))
    if S:
        segs.append((R, S, wg_b, wu_b, wd_b))

    mult_op = mybir.AluOpType.mult
    add_op = mybir.AluOpType.add

    with tile.TileContext(nc) as tc:
        with (
            tc.tile_pool(name="wpool", bufs=1) as wpool,
            tc.tile_pool(name="xpool", bufs=1) as xpool,
            tc.tile_pool(name="ypool", bufs=1) as ypool,
            tc.tile_pool(name="mpool", bufs=14) as mpool,
            tc.tile_pool(name="spool", bufs=3) as spool,
            tc.tile_pool(name="opool", bufs=3) as opool,
            tc.tile_pool(name="cpool", bufs=1) as cpool,
            tc.tile_pool(name="pg", bufs=1, space="PSUM") as pg,
            tc.tile_pool(name="pu", bufs=1, space="PSUM") as pu,
            tc.tile_pool(name="py", bufs=1, space="PSUM") as py,
        ):
            # --- PE warm-up + ACT table prewarm during initial DMA ---
            warm = cpool.tile([128, 128], F32, tag="warm")
            nc.gpsimd.memset(warm[:], 0.0)
            warm_ps = pg.tile([128, 128], F32, tag="g0", name="warm_ps")
            for i in range(56):
                nc.tensor.matmul(warm_ps[:], warm[:], warm[:],
                                 start=(i == 0), stop=(i == 55))
            warm_silu = cpool.tile([128, 128], F32, tag="warmsilu")
            nc.scalar.activation(warm_silu[:], warm[:],
                                 mybir.ActivationFunctionType.Silu)

            gate_sb = cpool.tile([128, NS], F32, tag="gate")
            first_dma = [True]

            for col0, Tseg, wg_d, wu_d, wd_d in segs:
                bsizes = _split_blocks(Tseg)
                boffs = [sum(bsizes[:i]) for i in range(len(bsizes))]
                nb = len(bsizes)
                # pair consecutive blocks (shared stationary, alternating
                # PSUM banks); a leftover block runs solo
                pairs = [(i, i + 1) for i in range(0, nb - 1, 2)]
                solos = [nb - 1] if nb % 2 else []

                # resident token blocks for this segment
                x_sb = [[xpool.tile([128, bsizes[j]], F32R, tag=f"x{c}b{j}",
                                    name=f"x{c}b{j}", padded_shape=[128, 512])
                         for j in range(nb)] for c in range(CT)]
                # y accumulator (per token subtile)
                y_acc = [ypool.tile([128, N_EMBD], F32, tag=f"ya{j}",
                                    name=f"yacc{j}")
                         for j in range(Tseg // 128)]

                def load_x(js):
                    for c in range(CT):
                        for j in js:
                            nc.sync.dma_start(
                                out=x_sb[c][j][:],
                                in_=xT[c * 128:(c + 1) * 128,
                                       bass.ds(col0 + boffs[j], bsizes[j])])

                for q in range(4):
                    h0 = q * HQ
                    with nc.named_scope(f"s{col0 > 0:d}q{q}"):
                        wg_sb = [wpool.tile([128, HQ], F32R, tag=f"wg{c}",
                                            name=f"wg{c}", bufs=2)
                                 for c in range(CT)]
                        wu_sb = [wpool.tile([128, HQ], F32R, tag=f"wu{c}",
                                            name=f"wu{c}") for c in range(CT)]
                        wd_sb = [wpool.tile([128, N_EMBD], F32R, tag=f"wd{h}",
                                            name=f"wd{h}") for h in range(HTQ)]
                        for c in range(CT):
                            nc.sync.dma_start(
                                out=wg_sb[c][:],
                                in_=wg_d[c * 128:(c + 1) * 128, h0:h0 + HQ])
                        if q == 0:
                            load_x(range(min(nb, 2)))
                        for c in range(CT):
                            nc.sync.dma_start(
                                out=wu_sb[c][:],
                                in_=wu_d[c * 128:(c + 1) * 128, h0:h0 + HQ])
                        for h in range(HTQ):
                            nc.sync.dma_start(
                                out=wd_sb[h][:],
                                in_=wd_d[h0 + h * 128:h0 + (h + 1) * 128, :])
                        if q == 0:
                            load_x(range(2, nb))
                        if first_dma[0]:
                            nc.sync.dma_start(out=gate_sb[:], in_=gate)
                            first_dma[0] = False

                        def yphase(j, mult):
                            # token-stationary down-projection for block j
                            for t in range(bsizes[j] // 128):
                                st = (col0 + boffs[j]) // 128 + t
                                sl = st - col0 // 128
                                y_ps = py.tile([128, N_EMBD], F32,
                                               tag=f"yt{t % 2}",
                                               name=f"yps{t % 2}")
                                for h in range(HTQ):
                                    lhsT = mult[h][:, bass.ts(t, 128)]
                                    for half in range(2):
                                        nc.tensor.matmul(
                                            y_ps[:, bass.ts(half, 512)],
                                            lhsT,
                                            wd_sb[h][:, bass.ts(half, 512)],
                                            start=(h == 0),
                                            stop=(h == HTQ - 1))
                                gate_ap = gate_sb[:, st:st + 1]
                                if q == 0:
                                    nc.vector.tensor_scalar_mul(
                                        y_acc[sl][:], y_ps[:], gate_ap)
                                elif q == 3:
                                    o = opool.tile([128, N_EMBD], F32, tag="o")
                                    nc.vector.scalar_tensor_tensor(
                                        o[:], y_ps[:], gate_ap, y_acc[sl][:],
                                        mult_op, add_op)
                                    nc.sync.dma_start(
                                        out=yO[st * 128:(st + 1) * 128, :],
                                        in_=o[:])
                                else:
                                    nc.vector.scalar_tensor_tensor(
                                        y_acc[sl][:], y_ps[:], gate_ap,
                                        y_acc[sl][:], mult_op, add_op)

                        def mults_for(blocks, h):
                            # g/u chains for 1 or 2 blocks; stationary shared
                            # across the pair so N=512 matmuls alternate banks
                            gps = [pg.tile([128, bsizes[j]], F32,
                                           tag=f"g{k}", name=f"g{k}",
                                           padded_shape=[128, 512])
                                   for k, j in enumerate(blocks)]
                            ups = [pu.tile([128, bsizes[j]], F32,
                                           tag=f"u{k}", name=f"u{k}",
                                           padded_shape=[128, 512])
                                   for k, j in enumerate(blocks)]
                            for c in range(CT):
                                for k, j in enumerate(blocks):
                                    nc.tensor.matmul(
                                        gps[k][:], wg_sb[c][:, bass.ts(h, 128)],
                                        x_sb[c][j][:], start=(c == 0),
                                        stop=(c == CT - 1))
                            for c in range(CT):
                                for k, j in enumerate(blocks):
                                    nc.tensor.matmul(
                                        ups[k][:], wu_sb[c][:, bass.ts(h, 128)],
                                        x_sb[c][j][:], start=(c == 0),
                                        stop=(c == CT - 1))
                            out = []
                            for k, j in enumerate(blocks):
                                silu_sb = spool.tile([128, bsizes[j]], F32,
                                                     tag="silu", name="silu_sb",
                                                     padded_shape=[128, 512])
                                nc.scalar.activation(
                                    silu_sb[:], gps[k][:],
                                    mybir.ActivationFunctionType.Silu)
                                m = mpool.tile([128, bsizes[j]], F32R, tag="m",
                                               name=f"m{h}",
                                               padded_shape=[128, 512])
                                nc.vector.tensor_mul(m[:], silu_sb[:], ups[k][:])
                                out.append(m)
                            return out

                        for jA, jB in pairs:
                            multA, multB = [], []
                            for h in range(HTQ):
                                ms = mults_for((jA, jB), h)
                                multA.append(ms[0])
                                multB.append(ms[1])
                            yphase(jA, multA)
                            yphase(jB, multB)
                        for j in solos:
                            multS = []
                            for h in range(HTQ):
                                multS.append(mults_for((j,), h)[0])
                            yphase(j, multS)

    nc.compile()
    return nc


def _get_program(R: int, S: int):
    key = (R, S)
    if key not in _PLAN_CACHE:
        _PLAN_CACHE[key] = _build_program(R, S)
    return _PLAN_CACHE[key]


def kernel(x, gate_w, expert_bias, shared_wg, shared_wu, shared_wd,
           routed_wg, routed_wu, routed_wd):
    x = np.asarray(x, dtype=np.float32)
    gate_w = np.asarray(gate_w, dtype=np.float32)
    expert_bias = np.asarray(expert_bias, dtype=np.float32)
    Bx, Tx, C = x.shape
    N = Bx * Tx
    assert C == N_EMBD
    xf = x.reshape(N, C)

    # ---- routing (host side of the expert-parallel dispatch) ----
    logits = xf.astype(np.float64) @ np.asarray(gate_w, np.float64).T
    m = logits.max(axis=1, keepdims=True)
    e = np.exp(logits - m)
    probs = e / e.sum(axis=1, keepdims=True)  # [N, 7] f64
    top1 = np.argmax(probs + expert_bias[None, :].astype(np.float64), axis=1)
    gate_val = probs[np.arange(N), top1].astype(np.float32)

    counts = np.bincount(top1, minlength=N_ROUTED)
    order = np.argsort(top1, kind="stable")  # tokens grouped by expert
    starts = np.zeros(N_ROUTED + 1, np.int64)
    np.cumsum(counts, out=starts[1:])

    R = max(256, int(math.ceil(counts.max() / 128)) * 128)
    S = int(math.ceil((N - R) / (N_CORES * 128))) * 128 if N > R else 0
    T = R + S

    # ---- per-core slot assignment ----
    # idx[core, slot] -> source token; gate per slot; pad slots: idx 0, gate 0
    idx = np.zeros((N_CORES, T), np.int64)
    gates = np.zeros((N_CORES, T), np.float32)
    for e_i in range(N_ROUTED):
        toks = order[starts[e_i]:starts[e_i + 1]]
        idx[e_i, :counts[e_i]] = toks
        gates[e_i, :counts[e_i]] = gate_val[toks]
    # shared: core 7 segment A takes tokens [0:R), segment B on all cores
    sh_a = min(R, N)
    idx[7, :sh_a] = np.arange(sh_a)
    gates[7, :sh_a] = 1.0
    sh_b_counts = []
    pos = sh_a
    for c in range(N_CORES):
        cnt = min(S, N - pos) if pos < N else 0
        if cnt > 0:
            idx[c, R:R + cnt] = np.arange(pos, pos + cnt)
            gates[c, R:R + cnt] = 1.0
        sh_b_counts.append((pos, cnt))
        pos += cnt
    assert pos >= N, f"shared capacity too small: {pos} < {N}"

    # ---- build device inputs ----
    xfT_r = _round_fp32r(np.ascontiguousarray(xf.T))  # [C, N]
    shared_wg_r = _round_fp32r(shared_wg[0])
    shared_wu_r = _round_fp32r(shared_wu[0])
    shared_wd_r = _round_fp32r(shared_wd[0])
    routed_wg_r = _round_fp32r(routed_wg)
    routed_wu_r = _round_fp32r(routed_wu)
    routed_wd_r = _round_fp32r(routed_wd)

    in_maps = []
    for c in range(N_CORES):
        xT_c = np.ascontiguousarray(np.take(xfT_r, idx[c], axis=1))
        gate_c = np.ascontiguousarray(gates[c].reshape(T // 128, 128).T)
        if c < N_ROUTED:
            wg_a, wu_a, wd_a = routed_wg_r[c], routed_wu_r[c], routed_wd_r[c]
        else:
            wg_a, wu_a, wd_a = shared_wg_r, shared_wu_r, shared_wd_r
        in_maps.append({
            "xT": xT_c, "gate": gate_c,
            "wg_a": wg_a, "wu_a": wu_a, "wd_a": wd_a,
            "wg_b": shared_wg_r, "wu_b": shared_wu_r, "wd_b": shared_wd_r,
        })

    nc = _get_program(R, S)
    try:
        res = run_bass_kernel_spmd(nc, in_maps, list(range(N_CORES)))
    except Exception:
        # transient NRT exec faults have been observed; retry once
        import time as _time
        _time.sleep(5.0)
        res = run_bass_kernel_spmd(nc, in_maps, list(range(N_CORES)))
    global LAST_RESULTS
    LAST_RESULTS = res

    # ---- combine (outputs are token-major [T, C]) ----
    y = np.zeros((N, C), dtype=np.float32)
    for e_i in range(N_ROUTED):
        cnt = counts[e_i]
        if cnt:
            out_c = res.results[e_i]["yO"]  # [T, C]
            y[idx[e_i, :cnt]] = out_c[:cnt]
    # shared segment A (core 7)
    out7 = res.results[7]["yO"]
    y[:sh_a] += out7[:sh_a]
    # shared segment B
    for c in range(N_CORES):
        pos_c, cnt = sh_b_counts[c]
        if cnt:
            out_c = res.results[c]["yO"]
            y[pos_c:pos_c + cnt] += out_c[R:R + cnt]

    return y.reshape(Bx, Tx, C), np.float32(0.0)
